# revision 3
# baseline (speedup 1.0000x reference)
"""MHSA over 32 independent 512-token segments, segment-parallel across 8
NeuronCores (4 segments / 2048 tokens per core, zero cross-core traffic).

Per core, per segment s (tokens [512s, 512s+512) of this core's shard):
  x^T       via PE transpose                       [1024, 512]  (f32r)
  Q^T,K^T   = (x @ Wq|k)^T  via lhsT=W, rhs=x^T    16x [128, 512]
  V         = x @ Wv        via lhsT=x^T, rhs=Wv   natural [tok, 1024] -> bf16
  S^T       = K Q^T per head                       [512k, 512q] psum
  A^T       = exp(S^T/8)                           bf16 sbuf (no max-sub: |S|<~6)
  O'^T      = [V|1]^T A^T   (ones col -> row 64 = rowsum Z)   [65, 512] psum
  Y^T       = O'^T[0:64] * (1/Z broadcast via PE outer-product)
  out       = Y^T.T @ Wproj

All big matmuls run in float32r (1 cyc/row, ~1.5e-4 rel err measured);
A@V runs bf16. Multi-sync-wait instructions are split onto NoOps (walrus
in this container allows one sync wait on several instruction formats).
"""

import numpy as np

import concourse.bass as bass
import concourse.mybir as mybir
import concourse.tile as tile
from concourse.bass_utils import run_bass_kernel_spmd

F32 = mybir.dt.float32
F32R = mybir.dt.float32r
BF16 = mybir.dt.bfloat16
EXP = mybir.ActivationFunctionType.Exp

T, C, H, HD = 16384, 1024, 16, 64
NCORES = 8
TOK = T // NCORES          # 2048 tokens per core
SEG = 512                  # tokens per segment
NSEG = TOK // SEG          # 4 segments per core
SCALE = 1.0 / np.sqrt(HD)  # folded into exp()


def _split_multi_waits(nc):
    """Move extra sync waits onto same-engine NoOps (1-wait ISA limit)."""
    for fn in nc.m.functions:
        for bb in fn.blocks:
            out = []
            for inst in bb.instructions:
                si = inst.sync_info
                if si is not None and si.on_wait and len(si.on_wait) > 1:
                    waits = list(si.on_wait)
                    for j, w in enumerate(waits[:-1]):
                        nop = mybir.InstNoOp(name=f"{inst.name}-wsp{j}")
                        nop.engine = inst.engine
                        nop.sync_info = mybir.SyncInfo(on_wait=[w], on_update=[])
                        out.append(nop)
                    inst.sync_info = mybir.SyncInfo(
                        on_wait=[waits[-1]], on_update=list(si.on_update)
                    )
                out.append(inst)
            bb.instructions = out


def _build():
    nc = bass.Bass("TRN2", target_bir_lowering=False, debug=False)
    x = nc.dram_tensor("x_sh", [TOK, C], F32, kind="ExternalInput").ap()
    wa = nc.dram_tensor("w_attn", [C, 3 * C], F32, kind="ExternalInput").ap()
    wp = nc.dram_tensor("w_proj", [C, C], F32, kind="ExternalInput").ap()
    out = nc.dram_tensor("out", [TOK, C], F32, kind="ExternalOutput").ap()

    ident_np = np.eye(128, dtype=np.float32)
    ones_np = np.ones((128, 64), dtype=np.float32)
    ident_d = nc.inline_tensor(ident_np, "ident_c").ap()
    ones_d = nc.inline_tensor(ones_np, "ones_c").ap()

    with tile.TileContext(nc) as tc:
        with (
            tc.tile_pool(name="const", bufs=1) as cpool,
            tc.tile_pool(name="wres", bufs=1) as wres,
            tc.tile_pool(name="stream", bufs=1) as stream,
            tc.tile_pool(name="work", bufs=1) as work,
            tc.tile_pool(name="ps", bufs=1, space="PSUM") as pspool,
            tc.tile_pool(name="dram", bufs=1, space="DRAM") as dpool,
        ):
            ps = pspool.tile([128, 4096], F32, tag="ps", name="ps")

            def bank(b, w=512, p0=0, pn=128):
                return ps[p0:pn, 512 * b:512 * b + w]

            ident = cpool.tile([128, 128], F32, tag="ident", name="ident")
            nc.sync.dma_start(ident[:], ident_d[:, :])
            ones_f = cpool.tile([128, 64], F32, tag="ones_f", name="ones_f")
            ones_r = cpool.tile([128, 64], F32R, tag="ones_r", name="ones_r")
            nc.sync.dma_start(ones_f[:], ones_d[:, :])
            nc.vector.tensor_copy(ones_r[:], ones_f[:])

            # ---- w_proj resident tiles (loaded during segment 0 phase A)
            wproj = [wres.tile([128, C], F32R, tag=f"wp{cc}", name=f"wp{cc}") for cc in range(8)]

            # ---- persistent per-segment working tiles
            xT = [work.tile([128, SEG], F32R, tag=f"xT{cc}", name=f"xT{cc}") for cc in range(8)]
            qkt = [work.tile([128, SEG], F32R, tag=f"qkt{m}", name=f"qkt{m}") for m in range(16)]
            vp = [work.tile([128, 16 * 66], BF16, tag=f"vp{q}", name=f"vp{q}") for q in range(4)]
            yt = [work.tile([128, SEG], F32R, tag=f"yt{r}", name=f"yt{r}") for r in range(8)]

            for s in range(NSEG):
                # ---------- phase A: load x, build x^T (banks 6-7 scratch)
                xns = []
                for qt in range(4):
                    xn = work.tile([128, C], F32, tag=f"xn{qt}", bufs=2, name=f"xn{s}_{qt}")
                    nc.sync.dma_start(xn[:], x[s * SEG + qt * 128: s * SEG + (qt + 1) * 128, :])
                    xns.append(xn)
                if s == 0:
                    for cc in range(8):
                        wtmp = stream.tile([128, C], F32, tag="wtmp", bufs=3, name=f"wt{cc}")
                        nc.sync.dma_start(wtmp[:], wp[cc * 128:(cc + 1) * 128, :])
                        nc.gpsimd.tensor_copy(wproj[cc][:], wtmp[:])
                for cc in range(8):
                    off = 3072 + (cc % 2) * 512
                    for qt in range(4):
                        nc.tensor.transpose(ps[0:128, off + qt * 128: off + (qt + 1) * 128],
                                            xns[qt][:, cc * 128:(cc + 1) * 128], ident[:])
                    nc.vector.tensor_copy(xT[cc][:], ps[0:128, off:off + 512])

                # ---------- phase B: Q^T, K^T (groups 0,1) then V (group 2)
                for g in range(2):
                    for cc in range(8):
                        wtm = stream.tile([128, C], F32, tag="wtmp", bufs=3, name=f"wf{s}_{g}_{cc}")
                        wrs = stream.tile([128, C], F32R, tag="wrs", bufs=3, name=f"w{s}_{g}_{cc}")
                        nc.sync.dma_start(wtm[:], wa[cc * 128:(cc + 1) * 128, g * C:(g + 1) * C])
                        nc.gpsimd.tensor_copy(wrs[:], wtm[:])
                        for m in range(8):
                            nc.tensor.matmul(
                                bank(m), wrs[:, m * 128:(m + 1) * 128], xT[cc][:],
                                start=(cc == 0), stop=(cc == 7),
                            )
                    for m in range(8):
                        nc.vector.tensor_copy(qkt[g * 8 + m][:], bank(m))

                for qt in range(4):
                    nc.vector.memset(
                        vp[qt].rearrange("p (h w) -> p h w", w=66)[:, :, 64:65], 1.0
                    )
                for cc in range(8):
                    wtm = stream.tile([128, C], F32, tag="wtmp", bufs=3, name=f"wv{s}_{cc}")
                    wrs = stream.tile([128, C], F32R, tag="wrs", bufs=3, name=f"w{s}_2_{cc}")
                    nc.sync.dma_start(wtm[:], wa[cc * 128:(cc + 1) * 128, 2 * C:3 * C])
                    nc.gpsimd.tensor_copy(wrs[:], wtm[:])
                    for qt in range(4):
                        for vn in range(2):
                            nc.tensor.matmul(
                                bank(qt * 2 + vn), xT[cc][:, qt * 128:(qt + 1) * 128],
                                wrs[:, vn * 512:(vn + 1) * 512],
                                start=(cc == 0), stop=(cc == 7),
                            )
                for qt in range(4):
                    for vn in range(2):
                        nc.vector.tensor_copy(
                            vp[qt].rearrange("p (h w) -> p h w", w=66)[:, vn * 8:(vn + 1) * 8, 0:64],
                            bank(qt * 2 + vn).rearrange("p (h w) -> p h w", w=64),
                        )

                # ---------- phase C: attention per head
                for h in range(16):
                    par = h % 2
                    qk_q = qkt[h // 2]
                    qk_k = qkt[8 + h // 2]
                    r0 = par * 64
                    for kt in range(4):
                        nc.tensor.matmul(
                            bank(kt), qk_k[r0:r0 + 64, kt * 128:(kt + 1) * 128],
                            qk_q[r0:r0 + 64, :], start=True, stop=True,
                        )
                    at0 = work.tile([128, 1024], BF16, tag="at0", bufs=2, name=f"at0_{s}_{h}")
                    at1 = work.tile([128, 1024], BF16, tag="at1", bufs=2, name=f"at1_{s}_{h}")
                    nc.scalar.activation(at0[:], ps[0:128, 0:1024], EXP, scale=SCALE)
                    nc.scalar.activation(at1[:], ps[0:128, 1024:2048], EXP, scale=SCALE)
                    ob = 2048 + par * 512
                    for kt in range(4):
                        at = at0 if kt < 2 else at1
                        nc.tensor.matmul(
                            ps[0:65, ob:ob + 512], vp[kt][:, 66 * h:66 * h + 65],
                            at[:, (kt % 2) * 512:(kt % 2 + 1) * 512],
                            start=(kt == 0), stop=(kt == 3),
                        )
                    # 1/Z at partition 64, broadcast to partitions 0-63 via PE
                    zf = work.tile([65, 512], F32, tag="zf", bufs=2, name=f"zf{s}_{h}")
                    zr = work.tile([65, 512], F32R, tag="zr", bufs=2, name=f"zr{s}_{h}")
                    nc.vector.reciprocal(zf[64:65, :], ps[64:65, ob:ob + 512])
                    nc.vector.tensor_copy(zr[64:65, :], zf[64:65, :])
                    rb = 3072 + par * 512
                    nc.tensor.matmul(
                        ps[0:64, rb:rb + 512], ones_r[64:65, :], zr[64:65, :],
                        start=True, stop=True,
                    )
                    rs = work.tile([64, 512], F32, tag="rs", bufs=2, name=f"rs{s}_{h}")
                    nc.scalar.copy(rs[:], ps[0:64, rb:rb + 512])
                    r = h // 2
                    if par == 0:
                        nc.vector.tensor_mul(yt[r][0:64, :], ps[0:64, ob:ob + 512], rs[:])
                    else:
                        tmp = work.tile([64, 512], F32R, tag="tmp", bufs=2, name=f"tm{s}_{h}")
                        nc.vector.tensor_mul(tmp[:], ps[0:64, ob:ob + 512], rs[:])
                        nc.sync.dma_start(yt[r][64:128, :], tmp[:])

                # ---------- phase D: projection
                for m in range(4):
                    for vn in range(2):
                        for cc in range(8):
                            nc.tensor.matmul(
                                bank(m * 2 + vn), yt[cc][:, m * 128:(m + 1) * 128],
                                wproj[cc][:, vn * 512:(vn + 1) * 512],
                                start=(cc == 0), stop=(cc == 7),
                            )
                for m in range(4):
                    ob_t = work.tile([128, C], F32, tag="ob", bufs=2, name=f"ob{s}_{m}")
                    for vn in range(2):
                        nc.scalar.copy(ob_t[:, vn * 512:(vn + 1) * 512], bank(m * 2 + vn))
                    nc.sync.dma_start(out[s * SEG + m * 128: s * SEG + (m + 1) * 128, :], ob_t[:])

    _split_multi_waits(nc)
    return nc


_NC = None


def kernel(x, w_attn, w_proj, split_sections):
    global _NC
    if _NC is None:
        _NC = _build()
    x = np.ascontiguousarray(np.asarray(x, dtype=np.float32))
    w_attn = np.ascontiguousarray(np.asarray(w_attn, dtype=np.float32))
    w_proj = np.ascontiguousarray(np.asarray(w_proj, dtype=np.float32))
    in_maps = [
        {"x_sh": x[i * TOK:(i + 1) * TOK], "w_attn": w_attn, "w_proj": w_proj}
        for i in range(NCORES)
    ]
    res = run_bass_kernel_spmd(_NC, in_maps, core_ids=list(range(NCORES)))
    return np.concatenate([res.results[i]["out"] for i in range(NCORES)], axis=0)


if __name__ == "__main__":
    rng = np.random.default_rng(0)
    x = rng.standard_normal((T, C), dtype=np.float32)
    wa = (rng.standard_normal((C, 3 * C), dtype=np.float32) / np.sqrt(C)).astype(np.float32)
    wpj = (rng.standard_normal((C, C), dtype=np.float32) / np.sqrt(C)).astype(np.float32)
    y = kernel(x, wa, wpj, np.arange(1, 32) * 512)
    print("out", y.shape, y.dtype, np.abs(y).mean())


# revision 7
# speedup vs baseline: 1.0403x; 1.0403x over previous
"""MHSA over 32 independent 512-token segments, segment-parallel across 8
NeuronCores (4 segments / 2048 tokens per core, zero cross-core traffic).

Per core, per segment s (tokens [512s, 512s+512) of this core's shard):
  x^T       via PE transpose                       [1024, 512]  (f32r)
  Q^T,K^T   = (x @ Wq|k)^T  via lhsT=W, rhs=x^T    16x [128, 512]
  V         = x @ Wv        via lhsT=x^T, rhs=Wv   natural [tok, 1024] -> bf16
  S^T       = K Q^T per head                       [512k, 512q] psum
  A^T       = exp(S^T/8)                           bf16 sbuf (no max-sub: |S|<~6)
  O'^T      = [V|1]^T A^T   (ones col -> row 64 = rowsum Z)   [65, 512] psum
  Y^T       = O'^T[0:64] * (1/Z broadcast via PE outer-product)
  out       = Y^T.T @ Wproj

All big matmuls run in float32r (1 cyc/row, ~1.5e-4 rel err measured);
A@V runs bf16. Multi-sync-wait instructions are split onto NoOps (walrus
in this container allows one sync wait on several instruction formats).
"""

import numpy as np

import concourse.bass as bass
import concourse.mybir as mybir
import concourse.tile as tile
from concourse.bass_utils import run_bass_kernel_spmd

F32 = mybir.dt.float32
F32R = mybir.dt.float32r
BF16 = mybir.dt.bfloat16
EXP = mybir.ActivationFunctionType.Exp

T, C, H, HD = 16384, 1024, 16, 64
NCORES = 8
TOK = T // NCORES          # 2048 tokens per core
SEG = 512                  # tokens per segment
NSEG = TOK // SEG          # 4 segments per core
SCALE = 1.0 / np.sqrt(HD)  # folded into exp()


def _split_multi_waits(nc):
    """Move extra sync waits onto same-engine NoOps (1-wait ISA limit)."""
    for fn in nc.m.functions:
        for bb in fn.blocks:
            out = []
            for inst in bb.instructions:
                si = inst.sync_info
                if si is not None and si.on_wait and len(si.on_wait) > 1:
                    waits = list(si.on_wait)
                    for j, w in enumerate(waits[:-1]):
                        nop = mybir.InstNoOp(name=f"{inst.name}-wsp{j}")
                        nop.engine = inst.engine
                        nop.sync_info = mybir.SyncInfo(on_wait=[w], on_update=[])
                        out.append(nop)
                    inst.sync_info = mybir.SyncInfo(
                        on_wait=[waits[-1]], on_update=list(si.on_update)
                    )
                out.append(inst)
            bb.instructions = out


def _build():
    nc = bass.Bass("TRN2", target_bir_lowering=False, debug=False)
    x = nc.dram_tensor("x_sh", [TOK, C], F32, kind="ExternalInput").ap()
    wa = nc.dram_tensor("w_attn", [C, 3 * C], F32, kind="ExternalInput").ap()
    wp = nc.dram_tensor("w_proj", [C, C], F32, kind="ExternalInput").ap()
    out = nc.dram_tensor("out", [TOK, C], F32, kind="ExternalOutput").ap()

    ident_np = np.eye(128, dtype=np.float32)
    ones_np = np.ones((128, 64), dtype=np.float32)
    ident_d = nc.inline_tensor(ident_np, "ident_c").ap()
    ones_d = nc.inline_tensor(ones_np, "ones_c").ap()

    with tile.TileContext(nc) as tc:
        with (
            tc.tile_pool(name="const", bufs=1) as cpool,
            tc.tile_pool(name="wres", bufs=1) as wres,
            tc.tile_pool(name="stream", bufs=1) as stream,
            tc.tile_pool(name="work", bufs=1) as work,
            tc.tile_pool(name="ps", bufs=1, space="PSUM") as pspool,
            tc.tile_pool(name="dram", bufs=1, space="DRAM") as dpool,
        ):
            ps = pspool.tile([128, 4096], F32, tag="ps", name="ps")

            def bank(b, w=512, p0=0, pn=128):
                return ps[p0:pn, 512 * b:512 * b + w]

            ident = cpool.tile([128, 128], F32, tag="ident", name="ident")
            nc.sync.dma_start(ident[:], ident_d[:, :])
            ones_f = cpool.tile([128, 64], F32, tag="ones_f", name="ones_f")
            ones_r = cpool.tile([128, 64], F32R, tag="ones_r", name="ones_r")
            nc.sync.dma_start(ones_f[:], ones_d[:, :])
            nc.vector.tensor_copy(ones_r[:], ones_f[:])

            # ---- w_proj resident tiles (loaded during segment 0 phase A)
            wproj = [wres.tile([128, C], F32R, tag=f"wp{cc}", name=f"wp{cc}") for cc in range(8)]

            # ---- persistent per-segment working tiles
            xT = [work.tile([128, SEG], F32R, tag=f"xT{cc}", name=f"xT{cc}") for cc in range(8)]
            qkt = [work.tile([128, SEG], F32R, tag=f"qkt{m}", name=f"qkt{m}") for m in range(16)]
            vp = [work.tile([128, 16 * 66], BF16, tag=f"vp{q}", name=f"vp{q}") for q in range(4)]
            yt = [work.tile([128, SEG], F32R, tag=f"yt{r}", name=f"yt{r}") for r in range(8)]

            for s in range(NSEG):
                # ---------- phase A: load x, build x^T (banks 6-7 scratch)
                xns = []
                for qt in range(4):
                    xn = work.tile([128, C], F32, tag=f"xn{qt}", bufs=2, name=f"xn{s}_{qt}")
                    nc.sync.dma_start(xn[:], x[s * SEG + qt * 128: s * SEG + (qt + 1) * 128, :])
                    xns.append(xn)
                if s == 0:
                    for cc in range(8):
                        wtmp = stream.tile([128, C], F32, tag="wtmp", bufs=4, name=f"wt{cc}")
                        nc.sync.dma_start(wtmp[:], wp[cc * 128:(cc + 1) * 128, :])
                        nc.gpsimd.tensor_copy(wproj[cc][:], wtmp[:])
                for cc in range(8):
                    off = 3072 + (cc % 2) * 512
                    for qt in range(4):
                        nc.tensor.transpose(ps[0:128, off + qt * 128: off + (qt + 1) * 128],
                                            xns[qt][:, cc * 128:(cc + 1) * 128], ident[:])
                    nc.vector.tensor_copy(xT[cc][:], ps[0:128, off:off + 512])

                # ---------- phase B: Q^T, K^T (groups 0,1) then V (group 2)
                for g in range(2):
                    for cc in range(8):
                        wtm = stream.tile([128, C], F32, tag="wtmp", bufs=4, name=f"wf{s}_{g}_{cc}")
                        wrs = stream.tile([128, C], F32R, tag="wrs", bufs=4, name=f"w{s}_{g}_{cc}")
                        nc.sync.dma_start(wtm[:], wa[cc * 128:(cc + 1) * 128, g * C:(g + 1) * C])
                        nc.gpsimd.tensor_copy(wrs[:], wtm[:])
                        for m in range(8):
                            nc.tensor.matmul(
                                bank(m), wrs[:, m * 128:(m + 1) * 128], xT[cc][:],
                                start=(cc == 0), stop=(cc == 7),
                            )
                    for m in range(8):
                        nc.vector.tensor_copy(qkt[g * 8 + m][:], bank(m))

                for qt in range(4):
                    nc.vector.memset(
                        vp[qt].rearrange("p (h w) -> p h w", w=66)[:, :, 64:65], 1.0
                    )
                for cc in range(8):
                    wtm = stream.tile([128, C], F32, tag="wtmp", bufs=4, name=f"wv{s}_{cc}")
                    wrs = stream.tile([128, C], F32R, tag="wrs", bufs=4, name=f"w{s}_2_{cc}")
                    nc.sync.dma_start(wtm[:], wa[cc * 128:(cc + 1) * 128, 2 * C:3 * C])
                    nc.gpsimd.tensor_copy(wrs[:], wtm[:])
                    for qt in range(4):
                        for vn in range(2):
                            nc.tensor.matmul(
                                bank(qt * 2 + vn), xT[cc][:, qt * 128:(qt + 1) * 128],
                                wrs[:, vn * 512:(vn + 1) * 512],
                                start=(cc == 0), stop=(cc == 7),
                            )
                for qt in range(4):
                    for vn in range(2):
                        nc.vector.tensor_copy(
                            vp[qt].rearrange("p (h w) -> p h w", w=66)[:, vn * 8:(vn + 1) * 8, 0:64],
                            bank(qt * 2 + vn).rearrange("p (h w) -> p h w", w=64),
                        )

                # ---------- phase C: attention per head
                for h in range(16):
                    par = h % 2
                    qk_q = qkt[h // 2]
                    qk_k = qkt[8 + h // 2]
                    r0 = par * 64
                    for kt in range(4):
                        nc.tensor.matmul(
                            bank(kt), qk_k[r0:r0 + 64, kt * 128:(kt + 1) * 128],
                            qk_q[r0:r0 + 64, :], start=True, stop=True,
                        )
                    at0 = work.tile([128, 2048], BF16, tag="at0", bufs=2, name=f"at0_{s}_{h}")
                    nc.scalar.activation(at0[:, 0:1024], ps[0:128, 0:1024], EXP, scale=SCALE)
                    nc.scalar.activation(at0[:, 1024:2048], ps[0:128, 1024:2048], EXP, scale=SCALE)
                    ob = 2048 + par * 512
                    for kt in range(4):
                        nc.tensor.matmul(
                            ps[0:65, ob:ob + 512], vp[kt][:, 66 * h:66 * h + 65],
                            at0[:, kt * 512:(kt + 1) * 512],
                            start=(kt == 0), stop=(kt == 3),
                        )
                    # 1/Z at partition 64, broadcast to partitions 0-63 via PE
                    zf = work.tile([65, 512], F32, tag="zf", bufs=2, name=f"zf{s}_{h}")
                    zr = work.tile([65, 512], F32R, tag="zr", bufs=2, name=f"zr{s}_{h}")
                    nc.vector.reciprocal(zf[64:65, :], ps[64:65, ob:ob + 512])
                    nc.vector.tensor_copy(zr[64:65, :], zf[64:65, :])
                    rb = 3072 + par * 512
                    nc.tensor.matmul(
                        ps[0:64, rb:rb + 512], ones_r[64:65, :], zr[64:65, :],
                        start=True, stop=True,
                    )
                    rs = work.tile([64, 512], F32, tag="rs", bufs=2, name=f"rs{s}_{h}")
                    nc.vector.tensor_copy(rs[:], ps[0:64, rb:rb + 512])
                    r = h // 2
                    if par == 0:
                        nc.vector.tensor_mul(yt[r][0:64, :], ps[0:64, ob:ob + 512], rs[:])
                    else:
                        tmp = work.tile([64, 512], F32R, tag="tmp", bufs=2, name=f"tm{s}_{h}")
                        nc.vector.tensor_mul(tmp[:], ps[0:64, ob:ob + 512], rs[:])
                        nc.sync.dma_start(yt[r][64:128, :], tmp[:])

                # ---------- phase D: projection
                for m in range(4):
                    ob_t = work.tile([128, C], F32, tag="ob", bufs=2, name=f"ob{s}_{m}")
                    for vn in range(2):
                        for cc in range(8):
                            nc.tensor.matmul(
                                bank((m * 2 + vn) % 6), yt[cc][:, m * 128:(m + 1) * 128],
                                wproj[cc][:, vn * 512:(vn + 1) * 512],
                                start=(cc == 0), stop=(cc == 7),
                            )
                        nc.scalar.copy(ob_t[:, vn * 512:(vn + 1) * 512], bank((m * 2 + vn) % 6))
                    nc.sync.dma_start(out[s * SEG + m * 128: s * SEG + (m + 1) * 128, :], ob_t[:])

    _split_multi_waits(nc)
    return nc


_NC = None


def kernel(x, w_attn, w_proj, split_sections):
    global _NC
    if _NC is None:
        _NC = _build()
    x = np.ascontiguousarray(np.asarray(x, dtype=np.float32))
    w_attn = np.ascontiguousarray(np.asarray(w_attn, dtype=np.float32))
    w_proj = np.ascontiguousarray(np.asarray(w_proj, dtype=np.float32))
    in_maps = [
        {"x_sh": x[i * TOK:(i + 1) * TOK], "w_attn": w_attn, "w_proj": w_proj}
        for i in range(NCORES)
    ]
    res = run_bass_kernel_spmd(_NC, in_maps, core_ids=list(range(NCORES)))
    return np.concatenate([res.results[i]["out"] for i in range(NCORES)], axis=0)


if __name__ == "__main__":
    rng = np.random.default_rng(0)
    x = rng.standard_normal((T, C), dtype=np.float32)
    wa = (rng.standard_normal((C, 3 * C), dtype=np.float32) / np.sqrt(C)).astype(np.float32)
    wpj = (rng.standard_normal((C, C), dtype=np.float32) / np.sqrt(C)).astype(np.float32)
    y = kernel(x, wa, wpj, np.arange(1, 32) * 512)
    print("out", y.shape, y.dtype, np.abs(y).mean())


# revision 8
# speedup vs baseline: 1.0821x; 1.0403x over previous
"""MHSA over 32 independent 512-token segments, segment-parallel across 8
NeuronCores (4 segments / 2048 tokens per core, zero cross-core traffic).

Per core, per segment s (tokens [512s, 512s+512) of this core's shard):
  x^T       via PE transpose                       [1024, 512]  (f32r)
  Q^T,K^T   = (x @ Wq|k)^T  via lhsT=W, rhs=x^T    16x [128, 512]
  V         = x @ Wv        via lhsT=x^T, rhs=Wv   natural [tok, 1024] -> bf16
  S^T       = K Q^T per head                       [512k, 512q] psum
  A^T       = exp(S^T/8)                           bf16 sbuf (no max-sub: |S|<~6)
  O'^T      = [V|1]^T A^T   (ones col -> row 64 = rowsum Z)   [65, 512] psum
  Y^T       = O'^T[0:64] * (1/Z broadcast via PE outer-product)
  out       = Y^T.T @ Wproj

All big matmuls run in float32r (1 cyc/row, ~1.5e-4 rel err measured);
A@V runs bf16. Multi-sync-wait instructions are split onto NoOps (walrus
in this container allows one sync wait on several instruction formats).
"""

import numpy as np

import concourse.bass as bass
import concourse.mybir as mybir
import concourse.tile as tile
from concourse.bass_utils import run_bass_kernel_spmd

F32 = mybir.dt.float32
F32R = mybir.dt.float32r
BF16 = mybir.dt.bfloat16
EXP = mybir.ActivationFunctionType.Exp

T, C, H, HD = 16384, 1024, 16, 64
NCORES = 8
TOK = T // NCORES          # 2048 tokens per core
SEG = 512                  # tokens per segment
NSEG = TOK // SEG          # 4 segments per core
SCALE = 1.0 / np.sqrt(HD)  # folded into exp()


def _split_multi_waits(nc):
    """Move extra sync waits onto same-engine NoOps (1-wait ISA limit)."""
    for fn in nc.m.functions:
        for bb in fn.blocks:
            out = []
            for inst in bb.instructions:
                si = inst.sync_info
                if si is not None and si.on_wait and len(si.on_wait) > 1:
                    waits = list(si.on_wait)
                    for j, w in enumerate(waits[:-1]):
                        nop = mybir.InstNoOp(name=f"{inst.name}-wsp{j}")
                        nop.engine = inst.engine
                        nop.sync_info = mybir.SyncInfo(on_wait=[w], on_update=[])
                        out.append(nop)
                    inst.sync_info = mybir.SyncInfo(
                        on_wait=[waits[-1]], on_update=list(si.on_update)
                    )
                out.append(inst)
            bb.instructions = out


def _build():
    nc = bass.Bass("TRN2", target_bir_lowering=False, debug=False)
    x = nc.dram_tensor("x_sh", [TOK, C], F32, kind="ExternalInput").ap()
    wa = nc.dram_tensor("w_attn", [C, 3 * C], F32, kind="ExternalInput").ap()
    wp = nc.dram_tensor("w_proj", [C, C], F32, kind="ExternalInput").ap()
    out = nc.dram_tensor("out", [TOK, C], F32, kind="ExternalOutput").ap()

    ident_np = np.eye(128, dtype=np.float32)
    ones_np = np.ones((128, 64), dtype=np.float32)
    ident_d = nc.inline_tensor(ident_np, "ident_c").ap()
    ones_d = nc.inline_tensor(ones_np, "ones_c").ap()

    with tile.TileContext(nc) as tc:
        with (
            tc.tile_pool(name="const", bufs=1) as cpool,
            tc.tile_pool(name="wres", bufs=1) as wres,
            tc.tile_pool(name="stream", bufs=1) as stream,
            tc.tile_pool(name="work", bufs=1) as work,
            tc.tile_pool(name="ps", bufs=1, space="PSUM") as pspool,
            tc.tile_pool(name="dram", bufs=1, space="DRAM") as dpool,
        ):
            ps = pspool.tile([128, 4096], F32, tag="ps", name="ps")

            def bank(b, w=512, p0=0, pn=128):
                return ps[p0:pn, 512 * b:512 * b + w]

            ident = cpool.tile([128, 128], F32, tag="ident", name="ident")
            nc.sync.dma_start(ident[:], ident_d[:, :])
            ones_f = cpool.tile([128, 64], F32, tag="ones_f", name="ones_f")
            ones_r = cpool.tile([128, 64], F32R, tag="ones_r", name="ones_r")
            nc.sync.dma_start(ones_f[:], ones_d[:, :])
            nc.vector.tensor_copy(ones_r[:], ones_f[:])

            # ---- w_proj resident tiles (loaded during segment 0 phase A)
            wproj = [wres.tile([128, C], F32R, tag=f"wp{cc}", name=f"wp{cc}") for cc in range(8)]

            # ---- persistent per-segment working tiles
            xT = [work.tile([128, SEG], F32R, tag=f"xT{cc}", name=f"xT{cc}") for cc in range(8)]
            qkt = [work.tile([128, SEG], F32R, tag=f"qkt{m}", name=f"qkt{m}") for m in range(16)]
            vp = [work.tile([128, 16 * 66], BF16, tag=f"vp{q}", name=f"vp{q}") for q in range(4)]
            yt = [work.tile([128, SEG], F32R, tag=f"yt{r}", name=f"yt{r}") for r in range(8)]

            for s in range(NSEG):
                # ---------- phase A: load x, build x^T (banks 6-7 scratch)
                xns = []
                for qt in range(4):
                    xn = work.tile([128, C], F32, tag=f"xn{qt}", bufs=1, name=f"xn{s}_{qt}")
                    nc.sync.dma_start(xn[:], x[s * SEG + qt * 128: s * SEG + (qt + 1) * 128, :])
                    xns.append(xn)
                for cc in range(8):
                    off = 3072 + (cc % 2) * 512
                    for qt in range(4):
                        nc.tensor.transpose(ps[0:128, off + qt * 128: off + (qt + 1) * 128],
                                            xns[qt][:, cc * 128:(cc + 1) * 128], ident[:])
                    nc.vector.tensor_copy(xT[cc][:], ps[0:128, off:off + 512])

                # ---------- phase B: Q^T, K^T (groups 0,1) then V (group 2)
                for g in range(2):
                    for cc in range(8):
                        wtm = stream.tile([128, C], F32, tag="wtmp", bufs=5, name=f"wf{s}_{g}_{cc}")
                        wrs = stream.tile([128, C], F32R, tag="wrs", bufs=5, name=f"w{s}_{g}_{cc}")
                        nc.sync.dma_start(wtm[:], wa[cc * 128:(cc + 1) * 128, g * C:(g + 1) * C])
                        nc.gpsimd.tensor_copy(wrs[:], wtm[:])
                        for m in range(8):
                            nc.tensor.matmul(
                                bank(m), wrs[:, m * 128:(m + 1) * 128], xT[cc][:],
                                start=(cc == 0), stop=(cc == 7),
                            )
                    for m in range(8):
                        nc.vector.tensor_copy(qkt[g * 8 + m][:], bank(m))

                for qt in range(4):
                    nc.vector.memset(
                        vp[qt].rearrange("p (h w) -> p h w", w=66)[:, :, 64:65], 1.0
                    )
                for cc in range(8):
                    wtm = stream.tile([128, C], F32, tag="wtmp", bufs=5, name=f"wv{s}_{cc}")
                    wrs = stream.tile([128, C], F32R, tag="wrs", bufs=5, name=f"w{s}_2_{cc}")
                    nc.sync.dma_start(wtm[:], wa[cc * 128:(cc + 1) * 128, 2 * C:3 * C])
                    nc.gpsimd.tensor_copy(wrs[:], wtm[:])
                    for qt in range(4):
                        for vn in range(2):
                            nc.tensor.matmul(
                                bank(qt * 2 + vn), xT[cc][:, qt * 128:(qt + 1) * 128],
                                wrs[:, vn * 512:(vn + 1) * 512],
                                start=(cc == 0), stop=(cc == 7),
                            )
                for qt in range(4):
                    for vn in range(2):
                        nc.vector.tensor_copy(
                            vp[qt].rearrange("p (h w) -> p h w", w=66)[:, vn * 8:(vn + 1) * 8, 0:64],
                            bank(qt * 2 + vn).rearrange("p (h w) -> p h w", w=64),
                        )

                if s == 0:
                    for cc in range(8):
                        wtmp = stream.tile([128, C], F32, tag="wtmp", bufs=5, name=f"wt{cc}")
                        nc.sync.dma_start(wtmp[:], wp[cc * 128:(cc + 1) * 128, :])
                        nc.gpsimd.tensor_copy(wproj[cc][:], wtmp[:])

                # ---------- phase C: attention per head
                for h in range(16):
                    par = h % 2
                    qk_q = qkt[h // 2]
                    qk_k = qkt[8 + h // 2]
                    r0 = par * 64
                    for kt in range(4):
                        nc.tensor.matmul(
                            bank(kt), qk_k[r0:r0 + 64, kt * 128:(kt + 1) * 128],
                            qk_q[r0:r0 + 64, :], start=True, stop=True,
                        )
                    at0 = work.tile([128, 2048], BF16, tag="at0", bufs=2, name=f"at0_{s}_{h}")
                    nc.scalar.activation(at0[:, 0:1024], ps[0:128, 0:1024], EXP, scale=SCALE)
                    nc.scalar.activation(at0[:, 1024:2048], ps[0:128, 1024:2048], EXP, scale=SCALE)
                    ob = 2048 + par * 512
                    for kt in range(4):
                        nc.tensor.matmul(
                            ps[0:65, ob:ob + 512], vp[kt][:, 66 * h:66 * h + 65],
                            at0[:, kt * 512:(kt + 1) * 512],
                            start=(kt == 0), stop=(kt == 3),
                        )
                    # 1/Z at partition 64, broadcast to partitions 0-63 via PE
                    zf = work.tile([65, 512], F32, tag="zf", bufs=2, name=f"zf{s}_{h}")
                    zr = work.tile([65, 512], F32R, tag="zr", bufs=2, name=f"zr{s}_{h}")
                    nc.vector.reciprocal(zf[64:65, :], ps[64:65, ob:ob + 512])
                    nc.vector.tensor_copy(zr[64:65, :], zf[64:65, :])
                    rb = 3072 + par * 512
                    nc.tensor.matmul(
                        ps[0:64, rb:rb + 512], ones_r[64:65, :], zr[64:65, :],
                        start=True, stop=True,
                    )
                    rs = work.tile([64, 512], F32, tag="rs", bufs=2, name=f"rs{s}_{h}")
                    nc.vector.tensor_copy(rs[:], ps[0:64, rb:rb + 512])
                    r = h // 2
                    if par == 0:
                        nc.vector.tensor_mul(yt[r][0:64, :], ps[0:64, ob:ob + 512], rs[:])
                    else:
                        tmp = work.tile([64, 512], F32R, tag="tmp", bufs=2, name=f"tm{s}_{h}")
                        nc.vector.tensor_mul(tmp[:], ps[0:64, ob:ob + 512], rs[:])
                        nc.sync.dma_start(yt[r][64:128, :], tmp[:])

                # ---------- phase D: projection
                for m in range(4):
                    ob_t = work.tile([128, C], F32, tag="ob", bufs=2, name=f"ob{s}_{m}")
                    for vn in range(2):
                        for cc in range(8):
                            nc.tensor.matmul(
                                bank((m * 2 + vn) % 6), yt[cc][:, m * 128:(m + 1) * 128],
                                wproj[cc][:, vn * 512:(vn + 1) * 512],
                                start=(cc == 0), stop=(cc == 7),
                            )
                        nc.scalar.copy(ob_t[:, vn * 512:(vn + 1) * 512], bank((m * 2 + vn) % 6))
                    nc.sync.dma_start(out[s * SEG + m * 128: s * SEG + (m + 1) * 128, :], ob_t[:])

    _split_multi_waits(nc)
    return nc


_NC = None


def kernel(x, w_attn, w_proj, split_sections):
    global _NC
    if _NC is None:
        _NC = _build()
    x = np.ascontiguousarray(np.asarray(x, dtype=np.float32))
    w_attn = np.ascontiguousarray(np.asarray(w_attn, dtype=np.float32))
    w_proj = np.ascontiguousarray(np.asarray(w_proj, dtype=np.float32))
    in_maps = [
        {"x_sh": x[i * TOK:(i + 1) * TOK], "w_attn": w_attn, "w_proj": w_proj}
        for i in range(NCORES)
    ]
    res = run_bass_kernel_spmd(_NC, in_maps, core_ids=list(range(NCORES)))
    return np.concatenate([res.results[i]["out"] for i in range(NCORES)], axis=0)


if __name__ == "__main__":
    rng = np.random.default_rng(0)
    x = rng.standard_normal((T, C), dtype=np.float32)
    wa = (rng.standard_normal((C, 3 * C), dtype=np.float32) / np.sqrt(C)).astype(np.float32)
    wpj = (rng.standard_normal((C, C), dtype=np.float32) / np.sqrt(C)).astype(np.float32)
    y = kernel(x, wa, wpj, np.arange(1, 32) * 512)
    print("out", y.shape, y.dtype, np.abs(y).mean())
